# revision 42
# baseline (speedup 1.0000x reference)
"""Trainium2 Bass kernel for CantorAttention (banded attention, fp8-residual
A-phase + transposed-PV softmax).

Per core (batch b, 4-head block hb; all tensors in Cantor-rank order):

  A:  QKV projection via fp8(e4m3) DoubleRow matmuls with residual
      compensation: x = x8 + xr8, w*64 = w8 + wr8 (residuals quantized into
      the denormal range -> absolute error ~2^-10 of parent, tighter than
      bf16).  psum = x8.w8 + x8.xr-terms at one shared scale 64; the 1/64
      folds into the psum->SBUF copy (DVE tensor_scalar / ACT scale port).
      4 DoubleRow passes replace 8 bf16 passes (2x PE).  A(0) is emitted
      term-pass-major chasing the halved initial DMAs; dummy matmuls +
      a dummy Exp pre-ramp the PE p-state and the ACT table load.
  S:  per (head, 128-query tile): banded scores over the tile's 2-3 aligned
      128-key chunks (bf16, contraction 64), Exp on ACT into a contiguous
      per-tile E block, {0,1} route mask multiplied in (DVE h2/h3,
      Pool h0/h1).
  PV: transposed: po4[q, 4*65] += E_chunk^T . V65 (65 cols/chunk, full
      128-row out utilization); V65's ones column makes col 64 the softmax
      denominator.
  N:  DVE reciprocal [128,4] + one stride-0-broadcast tensor_tensor
      -> attn [q, h, d] bf16; PE transpose (identity moving, accumulated
      pair) -> attn_outT [hd, q]; single DVE copy out.
  D:  out projection (bf16, contraction 256), psum->SBUF copies split
      ACT/DVE, stores on SP/ACT queues; tail blocks split into 256-col and
      2-mm pieces so the last store chain is short.

Software pipeline: per step t: N_pre(t-2) [DVE], scores/exp/mask(t),
PV(t-1), N_post(t-2) [PE transpose], plus an A- or D-piece to keep PE fed
while ACT chews the exps.  PSUM: 5-bank shared pool (scores/A/D) + 2 PV +
1 transpose.

Sharding: batch x head-block -> 8 cores.  Host sums the 4 partial outT
blocks per batch, transposes, un-permutes, adds out/v biases.
"""

import sys

sys.path.insert(0, "/opt/trn_rl_repo")

import numpy as np

B, S, DIM = 2, 2048, 1024
HEADS, DH = 16, 64
K_NEI = 64
N_CORES = 8
HPC = 4            # heads per core
QT = 128           # query tile
NT = S // QT       # 16 query tiles
WSCALE = 64.0      # fp8 weight pre-scale

_CACHE = {}


def _cantor_val(seq_len, depth=8):
    pos = np.arange(seq_len, dtype=np.float64)
    x = pos / max(1, seq_len - 1)
    x = np.clip(x, 1e-6, 1.0 - 1e-6)
    val = np.zeros_like(x)
    factor = 0.5
    for _ in range(depth):
        xs = x * 3.0
        digit = np.floor(xs)
        x = xs - digit
        val = val + (digit == 2.0).astype(np.float64) * factor
        factor *= 0.5
    return np.clip(val, 0.0, 1.0)


def _geometry(routes):
    """Banded-window geometry: per query tile the 128-aligned key chunks
    [a[t], a[t]+nch[t]) covering all routed keys, plus the sequential E-store
    block offsets eoff[t] (in chunks)."""
    val = _cantor_val(S)
    pi = np.argsort(val, kind="stable").astype(np.int64)
    rank = np.empty(S, np.int64)
    rank[pi] = np.arange(S)
    kr = rank[np.asarray(routes, np.int64)][pi]      # [S, K] key ranks
    a = np.zeros(NT, np.int64)
    nch = np.zeros(NT, np.int64)
    for t in range(NT):
        lo = int(kr[t * QT:(t + 1) * QT].min())
        hi = int(kr[t * QT:(t + 1) * QT].max()) + 1
        a[t] = lo // 128
        nch[t] = -(-(hi - a[t] * 128) // 128)
        if nch[t] > 4:
            raise ValueError("routes structure incompatible with banded kernel")
    eoff = np.concatenate([[0], np.cumsum(nch)[:-1]])
    return pi, rank, kr, a, nch, eoff


def _build_module(a, nch, eoff, loop_n=1):
    from contextlib import nullcontext

    from concourse import bacc, tile, mybir

    f32 = mybir.dt.float32
    bf16 = mybir.dt.bfloat16
    f8 = mybir.dt.float8e4
    AF = mybir.ActivationFunctionType
    ALU = mybir.AluOpType
    DR = mybir.MatmulPerfMode.DoubleRow
    a = [int(v) for v in a]
    nch = [int(v) for v in nch]
    eoff = [int(v) for v in eoff]
    E_CH = eoff[-1] + nch[-1]              # total chunks (46)
    E_COLS = E_CH * 128

    nc = bacc.Bacc("TRN2", target_bir_lowering=False, debug=False)
    x8d = nc.dram_tensor("x8", [DIM, S], f8, kind="ExternalInput").ap()
    xr8d = nc.dram_tensor("xr8", [DIM, S], f8, kind="ExternalInput").ap()
    wqk8d = nc.dram_tensor("wqk8", [DIM, 512], f8, kind="ExternalInput").ap()
    wqkr8d = nc.dram_tensor("wqkr8", [DIM, 512], f8, kind="ExternalInput").ap()
    wv8d = nc.dram_tensor("wv8", [DIM, 256], f8, kind="ExternalInput").ap()
    wvr8d = nc.dram_tensor("wvr8", [DIM, 256], f8, kind="ExternalInput").ap()
    bqkd = nc.dram_tensor("bqk", [512, 1], f32, kind="ExternalInput").ap()
    wod = nc.dram_tensor("wo", [256, DIM], bf16, kind="ExternalInput").ap()
    maskTd = nc.dram_tensor("maskT", [128, E_COLS], bf16, kind="ExternalInput").ap()
    identd = nc.dram_tensor("ident", [128, 128], bf16, kind="ExternalInput").ap()
    outp = nc.dram_tensor("outp", [DIM, S], bf16, kind="ExternalOutput").ap()

    r8 = lambda t: t.rearrange("(kk p) n -> p kk n", p=128)

    with tile.TileContext(nc) as tc:
        with tc.tile_pool(name="persist", bufs=1) as pp:
            # Tiles; DMA issue order is arranged inside the body so the
            # first x chunk leads the scalar queue and weights lead SP.
            bq_sb = pp.tile([128, 4], f32)
            ident_sb = pp.tile([128, 128], bf16)
            wqk8_sb = pp.tile([128, 8, 512], f8)
            wqkr8_sb = pp.tile([128, 8, 512], f8)
            wv8_sb = pp.tile([128, 8, 256], f8)
            wvr8_sb = pp.tile([128, 8, 256], f8)
            maskT_sb = pp.tile([128, E_COLS], bf16)
            wo_sb2 = pp.tile([128, 2, DIM], bf16)

            nc.sync.dma_start(out=wqk8_sb[:, 0:4, :], in_=r8(wqk8d)[:, 0:4, :])
            nc.sync.dma_start(out=wqk8_sb[:, 4:8, :], in_=r8(wqk8d)[:, 4:8, :])
            nc.sync.dma_start(out=wqkr8_sb[:, 0:4, :], in_=r8(wqkr8d)[:, 0:4, :])
            nc.sync.dma_start(out=wv8_sb, in_=r8(wv8d))
            nc.sync.dma_start(out=wqkr8_sb[:, 4:8, :], in_=r8(wqkr8d)[:, 4:8, :])
            nc.sync.dma_start(out=wvr8_sb, in_=r8(wvr8d))

            qk_sb = [pp.tile([128, S], bf16, tag=f"qk{m}", name=f"qk{m}")
                     for m in range(4)]
            V65 = pp.tile([128, NT, HPC, 65], bf16, tag="V65", name="V65")
            E_st = pp.tile([128, HPC, E_COLS], bf16, tag="Est", name="Est")
            attn_outT = pp.tile([128, 2, S], bf16, tag="aout", name="aout")
            nc.gpsimd.memset(V65[:, :, :, 64:65], 1.0)

            loop_cm = tc.For_i(0, loop_n, 1) if loop_n > 1 else nullcontext()
            with loop_cm:
                with tc.tile_pool(name="xt_pool", bufs=2) as pax, \
                     tc.tile_pool(name="st_pool", bufs=5) as pst, \
                     tc.tile_pool(name="rec_pool", bufs=3) as prc, \
                     tc.tile_pool(name="at_pool", bufs=4) as pat, \
                     tc.tile_pool(name="psB", bufs=6, space="PSUM") as psb, \
                     tc.tile_pool(name="psPV", bufs=1, space="PSUM") as pspv, \
                     tc.tile_pool(name="psTR", bufs=1, space="PSUM") as pstr:

                    po4_hold = {}
                    tr_hold = {}

                    # warmups during the initial DMA window: ramp the PE
                    # p-state and pull LoadActFuncSet off the hot path
                    dummy_sb = pp.tile([128, 512], bf16, tag="warm",
                                       name="warm")
                    nc.gpsimd.memset(dummy_sb, 0.0)
                    nc.scalar.activation(out=dummy_sb[0:1, 0:1],
                                         in_=dummy_sb[0:1, 1:2], func=AF.Exp)
                    wps = pspv.tile([128, 512], f32, tag="po", name="warmps")
                    for wi in range(7):
                        _lbl("warm")
                        nc.tensor.matmul(wps, dummy_sb[:, 0:128], dummy_sb,
                                         start=(wi == 0), stop=(wi == 6),
                                         skip_group_check=True)

                    def emit_A_dma(n):
                        x8t = pax.tile([128, 8, 512], f8, tag="x8", name=f"x8_{n}")
                        xr8t = pax.tile([128, 8, 512], f8, tag="xr8",
                                        name=f"xr8_{n}")
                        q_eng = nc.scalar if n == 0 else nc.sync
                        if n == 0:
                            # halves, in first-use order, so the term-major
                            # A(0) matmuls chase the DMA stream
                            q_eng.dma_start(out=x8t[:, 0:4, :],
                                            in_=r8(x8d)[:, 0:4, 0:512])
                            q_eng.dma_start(out=xr8t[:, 0:4, :],
                                            in_=r8(xr8d)[:, 0:4, 0:512])
                            q_eng.dma_start(out=x8t[:, 4:8, :],
                                            in_=r8(x8d)[:, 4:8, 0:512])
                            q_eng.dma_start(out=xr8t[:, 4:8, :],
                                            in_=r8(xr8d)[:, 4:8, 0:512])
                        else:
                            q_eng.dma_start(out=x8t,
                                            in_=r8(x8d)[:, :, n * 512:(n + 1) * 512])
                            q_eng.dma_start(out=xr8t,
                                            in_=r8(xr8d)[:, :, n * 512:(n + 1) * 512])
                        if n == 0:
                            nc.scalar.dma_start(
                                out=bq_sb,
                                in_=bqkd.rearrange("(m p) o -> p (m o)", p=128))
                        if n == 1:
                            nc.sync.dma_start(out=maskT_sb, in_=maskTd)
                            nc.sync.dma_start(out=ident_sb, in_=identd)
                        if n == 3:
                            nc.sync.dma_start(
                                out=wo_sb2,
                                in_=wod.rearrange("(p2 p) n -> p p2 n", p=128))
                        return x8t, xr8t

                    xh = {}

                    def emit_A_qk(n, m):
                        x8t, xr8t = xh[n]
                        terms_qk = ((wqk8_sb, x8t), (wqk8_sb, xr8t),
                                    (wqkr8_sb, x8t))
                        ps = psb.tile([128, 512], f32, tag="big",
                                      name=f"psqk{m}_{n}")
                        i = 0
                        for wt, xt in terms_qk:
                            for p in range(4):
                                nc.tensor.matmul(
                                    ps,
                                    wt[:, 2 * p:2 * p + 2, m * 128:(m + 1) * 128],
                                    xt[:, 2 * p:2 * p + 2, :],
                                    start=(i == 0), stop=(i == 11),
                                    perf_mode=DR)
                                i += 1
                        if n == 1:
                            nc.scalar.activation(
                                out=qk_sb[m][:, n * 512:(n + 1) * 512],
                                in_=ps, func=AF.Identity,
                                bias=bq_sb[:, m:m + 1], scale=1.0 / WSCALE)
                        else:
                            nc.vector.tensor_scalar(
                                qk_sb[m][:, n * 512:(n + 1) * 512], ps,
                                1.0 / WSCALE, bq_sb[:, m:m + 1],
                                ALU.mult, ALU.add)

                    def emit_A_v(n, ss):
                        x8t, xr8t = xh[n]
                        terms_v = ((x8t, wv8_sb), (xr8t, wv8_sb), (x8t, wvr8_sb))
                        cc = n * 4 + ss
                        ps = psb.tile([128, 512], f32, tag="big",
                                      name=f"psv{cc}")
                        psv = ps[:, 0:256]
                        i = 0
                        for xt, wt in terms_v:
                            for p in range(4):
                                nc.tensor.matmul(
                                    psv,
                                    xt[:, 2 * p:2 * p + 2, ss * 128:(ss + 1) * 128],
                                    wt[:, 2 * p:2 * p + 2, :],
                                    start=(i == 0), stop=(i == 11),
                                    perf_mode=DR, skip_group_check=True)
                                i += 1
                        if n == 1:
                            nc.vector.tensor_scalar(
                                V65[:, cc, :, 0:64],
                                psv.rearrange("p (h d) -> p h d", h=4),
                                1.0 / WSCALE, None, ALU.mult)
                        else:
                            nc.scalar.activation(
                                out=V65[:, cc, :, 0:64],
                                in_=psv.rearrange("p (h d) -> p h d", h=4),
                                func=AF.Copy, scale=1.0 / WSCALE)

                    def emit_A0_termmajor():
                        x8t, xr8t = xh[0]
                        seq = [(0, 0), (0, 1), (1, 0), (1, 1), (2, 0), (2, 1),
                               (0, 2), (0, 3), (1, 2), (1, 3), (2, 2), (2, 3)]
                        pss_ = [psb.tile([128, 512], f32, tag="big",
                                         name=f"psqk{m}_0") for m in range(4)]
                        terms = ((wqk8_sb, x8t), (wqk8_sb, xr8t),
                                 (wqkr8_sb, x8t))
                        _lbl("Aqk0")
                        for si, (ti, p) in enumerate(seq):
                            wt, xt = terms[ti]
                            for m in range(4):
                                nc.tensor.matmul(
                                    pss_[m],
                                    wt[:, 2 * p:2 * p + 2, m * 128:(m + 1) * 128],
                                    xt[:, 2 * p:2 * p + 2, :],
                                    start=(si == 0), stop=(si == 11),
                                    perf_mode=DR)
                        for m in range(4):
                            nc.vector.tensor_scalar(
                                qk_sb[m][:, 0:512], pss_[m],
                                1.0 / WSCALE, bq_sb[:, m:m + 1],
                                ALU.mult, ALU.add)
                        for ss in range(4):
                            emit_A_v(0, ss)

                    def emit_A(n):
                        if n not in xh:
                            xh[n] = emit_A_dma(n)
                        for m in range(4):
                            emit_A_qk(n, m)
                        for ss in range(4):
                            emit_A_v(n, ss)

                    def emit_Ssc(t):
                        """scores + exp + mask for tile t."""
                        e0 = eoff[t] * 128
                        ncols = nch[t] * 128
                        for h in range(HPC):
                            poff = (h % 2) * 64
                            qT = qk_sb[h // 2]
                            kT = qk_sb[2 + h // 2]
                            ps = psb.tile([128, 512], f32, tag="big",
                                          name=f"sc{h}_{t}")
                            for j in range(nch[t]):
                                nc.tensor.matmul(
                                    ps[:, j * 128:(j + 1) * 128],
                                    kT[poff:poff + 64,
                                       (a[t] + j) * 128:(a[t] + j + 1) * 128],
                                    qT[poff:poff + 64, t * 128:(t + 1) * 128],
                                    start=(j == 0), stop=(j == nch[t] - 1),
                                    skip_group_check=True)
                            nc.scalar.activation(
                                out=E_st[:, h, e0:e0 + ncols],
                                in_=ps[:, 0:ncols], func=AF.Exp)
                            eng = nc.vector if h >= 2 else nc.gpsimd
                            eng.tensor_tensor(
                                E_st[:, h, e0:e0 + ncols],
                                E_st[:, h, e0:e0 + ncols],
                                maskT_sb[:, e0:e0 + ncols], ALU.mult)

                    def emit_PV(t):
                        e0 = eoff[t] * 128
                        po4 = pspv.tile([128, 512], f32, tag="po", name=f"po{t}")
                        po4_hold[t] = po4
                        nmm = HPC * nch[t]
                        i = 0
                        for h in range(HPC):
                            for j in range(nch[t]):
                                nc.tensor.matmul(
                                    po4[:, h * 65:h * 65 + 65],
                                    E_st[:, h, e0 + j * 128:e0 + (j + 1) * 128],
                                    V65[:, a[t] + j, h, :],
                                    start=(i == 0), stop=(i == nmm - 1),
                                    skip_group_check=True)
                                i += 1

                    au_hold = {}

                    def emit_N_pre(t):
                        """normalize (DVE) for tile t."""
                        from concourse.bass import AP

                        po4 = po4_hold.pop(t)
                        den = AP(po4.tensor, po4.offset + 64,
                                 [list(po4.ap[0]), [65, 4]])
                        rec = prc.tile([128, 4], f32, tag="rec", name=f"rec{t}")
                        nc.vector.reciprocal(rec, den)
                        au = pat.tile([128, HPC, 64], bf16, tag="at",
                                      name=f"at{t}")
                        au_hold[t] = au
                        un = AP(po4.tensor, po4.offset,
                                [list(po4.ap[0]), [65, 4], [1, 64]])
                        recb = AP(rec.tensor, rec.offset,
                                  [list(rec.ap[0]), [1, 4], [0, 64]])
                        nc.vector.tensor_tensor(au, un, recb, ALU.mult)

                    def emit_N_post(t):
                        """transpose + copy-out for tile t."""
                        au = au_hold.pop(t)
                        tr = pstr.tile([128, 1024], bf16, tag="tr",
                                       name=f"tr{t}")
                        for i in range(2):
                            _lbl(f"TR{t}")
                            nc.tensor.matmul(
                                tr[:, i * 128:(i + 1) * 128],
                                au[:, 2 * i:2 * i + 2, :], ident_sb,
                                is_transpose=True,
                                start=(i == 0), stop=(i == 1),
                                skip_group_check=True)
                        nc.vector.tensor_copy(
                            attn_outT[:, :, t * 128:(t + 1) * 128],
                            tr[:, 0:256].rearrange("p (i q) -> p i q", i=2))

                    def emit_N(t):
                        emit_N_pre(t)
                        emit_N_post(t)

                    outp3 = outp.rearrange("(mm p) n -> p mm n", p=128)

                    def emit_D_cols_g(nb, ci, g, eng):
                        """4-mm piece over a 256-col half-block (tail)."""
                        c0 = nb * 512 + ci * 256
                        st = pst.tile([128, 4, 256], bf16, tag="st3",
                                      name=f"st3_{nb}_{ci}_{g}")
                        for i in range(4):
                            mm = g * 4 + i
                            _lbl(f"D{nb}.{mm}c{ci}")
                            ps = psb.tile([128, 512], f32, tag="big",
                                          name=f"psd{mm}_{nb}c{ci}")
                            psx = ps[:, 0:256]
                            for p2 in range(2):
                                nc.tensor.matmul(
                                    psx,
                                    wo_sb2[:, p2, mm * 128:(mm + 1) * 128],
                                    attn_outT[:, p2, c0:c0 + 256],
                                    start=(p2 == 0), stop=(p2 == 1),
                                    skip_group_check=True)
                            eng(st[:, i, :], psx)
                        q_eng = nc.scalar if (g + ci) % 2 else nc.sync
                        q_eng.dma_start(
                            out=outp3[:, g * 4:(g + 1) * 4, c0:c0 + 256],
                            in_=st)

                    def emit_D_cols2(nb, ci, g, sub):
                        """2-mm tail piece: parallel ACT/DVE copies + store."""
                        c0 = nb * 512 + ci * 256
                        m0 = g * 4 + sub * 2
                        st = pst.tile([128, 2, 256], bf16, tag="st4",
                                      name=f"st4_{nb}_{ci}_{g}_{sub}")
                        for idx in range(2):
                            mm = m0 + idx
                            _lbl(f"D{nb}.{mm}c{ci}")
                            ps = psb.tile([128, 512], f32, tag="big",
                                          name=f"psd{mm}_{nb}c{ci}")
                            psx = ps[:, 0:256]
                            for p2 in range(2):
                                nc.tensor.matmul(
                                    psx,
                                    wo_sb2[:, p2, mm * 128:(mm + 1) * 128],
                                    attn_outT[:, p2, c0:c0 + 256],
                                    start=(p2 == 0), stop=(p2 == 1),
                                    skip_group_check=True)
                            if idx == 0:
                                nc.scalar.copy(st[:, idx, :], psx)
                            else:
                                nc.vector.tensor_copy(st[:, idx, :], psx)
                        q_eng = nc.scalar if sub % 2 else nc.sync
                        q_eng.dma_start(
                            out=outp3[:, m0:m0 + 2, c0:c0 + 256], in_=st)

                    def emit_D_half(nb, half):
                        if nb == 3:
                            emit_D_pair(nb, half, 0)
                            emit_D_pair(nb, half, 1)
                            return
                        st = pst.tile([128, 4, 512], bf16, tag="st",
                                      name=f"st{half}_{nb}")
                        for i in range(4):
                            mm = half * 4 + i
                            _lbl(f"D{nb}.{mm}")
                            ps = psb.tile([128, 512], f32, tag="big",
                                          name=f"psd{mm}_{nb}")
                            for p2 in range(2):
                                nc.tensor.matmul(
                                    ps, wo_sb2[:, p2, mm * 128:(mm + 1) * 128],
                                    attn_outT[:, p2, nb * 512:(nb + 1) * 512],
                                    start=(p2 == 0), stop=(p2 == 1))
                            if (mm + nb) % 2 == 0:
                                nc.scalar.copy(st[:, i, :], ps)
                            else:
                                nc.vector.tensor_copy(st[:, i, :], ps)
                        nc.sync.dma_start(
                            out=outp3[:, half * 4:(half + 1) * 4,
                                      nb * 512:(nb + 1) * 512],
                            in_=st)

                    # software-pipelined emission: PV lags scores by one
                    # step, N by two; A/D matmul pieces fill PE while ACT
                    # chews the exps.
                    xh[0] = emit_A_dma(0)
                    emit_A0_termmajor()
                    emit_A(1)
                    xh[2] = emit_A_dma(2)
                    emit_Ssc(0)
                    emit_Ssc(1); emit_PV(0); emit_A_qk(2, 0)
                    emit_Ssc(2); emit_PV(1); emit_N(0); emit_A_qk(2, 1)
                    emit_N_pre(1); emit_Ssc(3); emit_PV(2); emit_N_post(1); emit_A_qk(2, 2)
                    emit_N_pre(2); emit_Ssc(4); emit_PV(3); emit_N_post(2); emit_A_qk(2, 3)
                    xh[3] = emit_A_dma(3)
                    emit_N_pre(3); emit_Ssc(5); emit_PV(4); emit_N_post(3); emit_A_v(2, 0); emit_A_v(2, 1)
                    emit_N_pre(4); emit_Ssc(6); emit_PV(5); emit_N_post(4); emit_A_v(2, 2); emit_A_v(2, 3)
                    emit_N_pre(5); emit_Ssc(7); emit_PV(6); emit_N_post(5); emit_A_qk(3, 0); emit_A_qk(3, 1)
                    emit_N_pre(6); emit_Ssc(8); emit_PV(7); emit_N_post(6); emit_A_qk(3, 2); emit_A_qk(3, 3)
                    emit_N_pre(7); emit_Ssc(9); emit_PV(8); emit_N_post(7); emit_A_v(3, 0); emit_A_v(3, 1)
                    emit_N_pre(8); emit_Ssc(10); emit_PV(9); emit_N_post(8); emit_A_v(3, 2); emit_A_v(3, 3)
                    emit_N_pre(9); emit_Ssc(11); emit_PV(10); emit_N_post(9); emit_D_half(0, 0)
                    emit_N_pre(10); emit_Ssc(12); emit_PV(11); emit_N_post(10); emit_D_half(0, 1)
                    emit_N_pre(11); emit_Ssc(13); emit_PV(12); emit_N_post(11); emit_D_half(1, 0)
                    emit_N_pre(12); emit_Ssc(14); emit_PV(13); emit_N_post(12); emit_D_half(1, 1)
                    emit_N_pre(13); emit_Ssc(15); emit_PV(14); emit_N_post(13); emit_D_half(2, 0)
                    emit_PV(15); emit_D_half(2, 1)
                    emit_N(14)
                    emit_D_cols_g(3, 0, 0, lambda o, i: nc.scalar.copy(o, i))
                    emit_N(15)
                    emit_D_cols_g(3, 0, 1, lambda o, i: nc.vector.tensor_copy(o, i))
                    emit_D_cols_g(3, 1, 0, lambda o, i: nc.scalar.copy(o, i))
                    emit_D_cols2(3, 1, 1, 0)
                    emit_D_cols2(3, 1, 1, 1)

    nc.compile()
    return nc


def _get_module(a, nch, eoff):
    key = (tuple(int(v) for v in a), tuple(int(v) for v in nch))
    if key not in _CACHE:
        _CACHE[key] = _build_module(a, nch, eoff)
    return _CACHE[key]


def _split8(arr):
    """x -> (x8, xr8) e4m3 pair with x ~= x8 + xr8 (residual hits denormals)."""
    import ml_dtypes

    f8 = ml_dtypes.float8_e4m3
    a = np.asarray(arr, np.float32)
    hi = a.astype(f8)
    lo = (a - hi.astype(np.float32)).astype(f8)
    return hi, lo


def _prepare_in_maps(x, routes, qkv_w, qkv_b, out_w, out_b):
    import ml_dtypes

    bf = ml_dtypes.bfloat16
    x = np.ascontiguousarray(np.asarray(x, np.float32))
    qkv_w = np.asarray(qkv_w, np.float32)
    qkv_b = np.asarray(qkv_b, np.float32)
    out_w = np.asarray(out_w, np.float32)

    pi, rank, kr, a, nch, eoff = _geometry(np.asarray(routes))
    E_CH = int(eoff[-1] + nch[-1])
    E_COLS = E_CH * 128
    SCALE = 1.0 / float(np.sqrt(DH))

    # maskT [128, E_COLS]: maskT[p, (eoff[t]+j)*128 + q] = 1 iff key
    # (a[t]+j)*128+p is routed for query t*128+q
    maskT_np = np.zeros((128, E_COLS), np.float32)
    for t in range(NT):
        krt = kr[t * QT:(t + 1) * QT] - a[t] * 128          # [128, K]
        qi = np.repeat(np.arange(QT), K_NEI)
        kk = krt.ravel()
        j, p = kk // 128, kk % 128
        maskT_np[p, (eoff[t] + j) * 128 + qi] = 1.0
    maskT_np = maskT_np.astype(bf)

    ident_np = np.eye(128, dtype=np.float32).astype(bf)

    x8_b, xr8_b = [], []
    for b in range(B):
        x8, xr8 = _split8(x[b][pi].T)
        x8_b.append(np.ascontiguousarray(x8))
        xr8_b.append(np.ascontiguousarray(xr8))

    in_maps = []
    for c in range(N_CORES):
        b = c // (N_CORES // B)
        hb = c % (N_CORES // B)
        heads = range(hb * HPC, (hb + 1) * HPC)
        w_cols, b_rows = [], []
        for sect, scale in ((0, SCALE), (1, 1.0)):
            for h in heads:
                r0 = sect * DIM + h * DH
                w_cols.append(qkv_w[r0:r0 + DH] * scale)
                b_rows.append(qkv_b[r0:r0 + DH] * scale)
        wqk = np.concatenate(w_cols, 0).T * WSCALE           # [1024, 512]
        wqk8, wqkr8 = _split8(wqk)
        bqk_c = np.concatenate(b_rows, 0).reshape(-1, 1).astype(np.float32)
        vr0 = 2 * DIM + hb * HPC * DH
        wv = qkv_w[vr0:vr0 + 256].T * WSCALE                 # [1024, 256]
        wv8, wvr8 = _split8(wv)
        wo_c = np.ascontiguousarray(
            out_w[:, hb * HPC * DH:(hb + 1) * HPC * DH].T).astype(bf)
        in_maps.append({
            "x8": x8_b[b], "xr8": xr8_b[b],
            "wqk8": np.ascontiguousarray(wqk8),
            "wqkr8": np.ascontiguousarray(wqkr8),
            "wv8": np.ascontiguousarray(wv8),
            "wvr8": np.ascontiguousarray(wvr8),
            "bqk": bqk_c,
            "wo": wo_c,
            "maskT": maskT_np,
            "ident": ident_np,
        })
    return in_maps, pi, (a, nch, eoff)


def kernel(x, routes, qkv_w, qkv_b, out_w, out_b):
    from concourse.bass_utils import run_bass_kernel_spmd

    out_b = np.asarray(out_b, np.float32)
    qkv_b = np.asarray(qkv_b, np.float32)
    out_w = np.asarray(out_w, np.float32)
    in_maps, pi, geom = _prepare_in_maps(x, routes, qkv_w, qkv_b, out_w, out_b)

    nc = _get_module(*geom)
    res = run_bass_kernel_spmd(nc, in_maps, core_ids=list(range(N_CORES)))

    # v-bias: probs sum to 1 -> attn@(v+bv) = attn@v + bv; outp picks up a
    # constant wo.T @ bv per core -- add on host (in the kernel's bf16 wo).
    bv_all = qkv_b[2 * DIM:3 * DIM]
    adj = np.zeros(DIM, np.float64)
    for hb in range(N_CORES // B):
        sl = slice(hb * HPC * DH, (hb + 1) * HPC * DH)
        wo_c = np.asarray(in_maps[hb]["wo"], np.float64)     # [256, DIM]
        adj += bv_all[sl].astype(np.float64) @ wo_c

    out = np.empty((B, S, DIM), np.float32)
    for b in range(B):
        cores = [c for c in range(N_CORES) if c // (N_CORES // B) == b]
        outT = res.results[cores[0]]["outp"].astype(np.float32)
        for c in cores[1:]:
            outT = outT + res.results[c]["outp"].astype(np.float32)
        rows_sorted = outT.T                      # [S, DIM] in rank order
        tmp = np.empty_like(rows_sorted)
        tmp[pi] = rows_sorted
        out[b] = tmp + (out_b.astype(np.float64) + adj)[None, :].astype(np.float32)
    return out


# revision 43
# speedup vs baseline: 1.0092x; 1.0092x over previous
"""Trainium2 Bass kernel for CantorAttention (banded attention, fp8-residual
A-phase + transposed-PV softmax).

Per core (batch b, 4-head block hb; all tensors in Cantor-rank order):

  A:  QKV projection via fp8(e4m3) DoubleRow matmuls with residual
      compensation: x = x8 + xr8, w*64 = w8 + wr8 (residuals quantized into
      the denormal range -> absolute error ~2^-10 of parent, tighter than
      bf16).  psum = x8.w8 + x8.xr-terms at one shared scale 64; the 1/64
      folds into the psum->SBUF copy (DVE tensor_scalar / ACT scale port).
      4 DoubleRow passes replace 8 bf16 passes (2x PE).  A(0) is emitted
      term-pass-major chasing the halved initial DMAs; dummy matmuls +
      a dummy Exp pre-ramp the PE p-state and the ACT table load.
  S:  per (head, 128-query tile): banded scores over the tile's 2-3 aligned
      128-key chunks (bf16, contraction 64), Exp on ACT into a contiguous
      per-tile E block, {0,1} route mask multiplied in (DVE h2/h3,
      Pool h0/h1).
  PV: transposed: po4[q, 4*65] += E_chunk^T . V65 (65 cols/chunk, full
      128-row out utilization); V65's ones column makes col 64 the softmax
      denominator.
  N:  DVE reciprocal [128,4] + one stride-0-broadcast tensor_tensor
      -> attn [q, h, d] bf16; PE transpose (identity moving, accumulated
      pair) -> attn_outT [hd, q]; single DVE copy out.
  D:  out projection (bf16, contraction 256), psum->SBUF copies split
      ACT/DVE, stores on SP/ACT queues; tail blocks split into 256-col and
      2-mm pieces so the last store chain is short.

Software pipeline: per step t: N_pre(t-2) [DVE], scores/exp/mask(t),
PV(t-1), N_post(t-2) [PE transpose], plus an A- or D-piece to keep PE fed
while ACT chews the exps.  PSUM: 5-bank shared pool (scores/A/D) + 2 PV +
1 transpose.

Sharding: batch x head-block -> 8 cores.  Host sums the 4 partial outT
blocks per batch, transposes, un-permutes, adds out/v biases.
"""

import sys

sys.path.insert(0, "/opt/trn_rl_repo")

import numpy as np

B, S, DIM = 2, 2048, 1024
HEADS, DH = 16, 64
K_NEI = 64
N_CORES = 8
HPC = 4            # heads per core
QT = 128           # query tile
NT = S // QT       # 16 query tiles
WSCALE = 64.0      # fp8 weight pre-scale

_CACHE = {}


def _cantor_val(seq_len, depth=8):
    pos = np.arange(seq_len, dtype=np.float64)
    x = pos / max(1, seq_len - 1)
    x = np.clip(x, 1e-6, 1.0 - 1e-6)
    val = np.zeros_like(x)
    factor = 0.5
    for _ in range(depth):
        xs = x * 3.0
        digit = np.floor(xs)
        x = xs - digit
        val = val + (digit == 2.0).astype(np.float64) * factor
        factor *= 0.5
    return np.clip(val, 0.0, 1.0)


def _geometry(routes):
    """Banded-window geometry: per query tile the 128-aligned key chunks
    [a[t], a[t]+nch[t]) covering all routed keys, plus the sequential E-store
    block offsets eoff[t] (in chunks)."""
    val = _cantor_val(S)
    pi = np.argsort(val, kind="stable").astype(np.int64)
    rank = np.empty(S, np.int64)
    rank[pi] = np.arange(S)
    kr = rank[np.asarray(routes, np.int64)][pi]      # [S, K] key ranks
    a = np.zeros(NT, np.int64)
    nch = np.zeros(NT, np.int64)
    for t in range(NT):
        lo = int(kr[t * QT:(t + 1) * QT].min())
        hi = int(kr[t * QT:(t + 1) * QT].max()) + 1
        a[t] = lo // 128
        nch[t] = -(-(hi - a[t] * 128) // 128)
        if nch[t] > 4:
            raise ValueError("routes structure incompatible with banded kernel")
    eoff = np.concatenate([[0], np.cumsum(nch)[:-1]])
    return pi, rank, kr, a, nch, eoff


def _build_module(a, nch, eoff, loop_n=1):
    from contextlib import nullcontext

    from concourse import bacc, tile, mybir

    f32 = mybir.dt.float32
    bf16 = mybir.dt.bfloat16
    f8 = mybir.dt.float8e4
    AF = mybir.ActivationFunctionType
    ALU = mybir.AluOpType
    DR = mybir.MatmulPerfMode.DoubleRow
    a = [int(v) for v in a]
    nch = [int(v) for v in nch]
    eoff = [int(v) for v in eoff]
    E_CH = eoff[-1] + nch[-1]              # total chunks (46)
    E_COLS = E_CH * 128

    nc = bacc.Bacc("TRN2", target_bir_lowering=False, debug=False)
    x8d = nc.dram_tensor("x8", [DIM, S], f8, kind="ExternalInput").ap()
    xr8d = nc.dram_tensor("xr8", [DIM, S], f8, kind="ExternalInput").ap()
    wqk8d = nc.dram_tensor("wqk8", [DIM, 512], f8, kind="ExternalInput").ap()
    wqkr8d = nc.dram_tensor("wqkr8", [DIM, 512], f8, kind="ExternalInput").ap()
    wv8d = nc.dram_tensor("wv8", [DIM, 256], f8, kind="ExternalInput").ap()
    wvr8d = nc.dram_tensor("wvr8", [DIM, 256], f8, kind="ExternalInput").ap()
    bqkd = nc.dram_tensor("bqk", [512, 1], f32, kind="ExternalInput").ap()
    wod = nc.dram_tensor("wo", [256, DIM], bf16, kind="ExternalInput").ap()
    maskTd = nc.dram_tensor("maskT", [128, E_COLS], bf16, kind="ExternalInput").ap()
    identd = nc.dram_tensor("ident", [128, 128], bf16, kind="ExternalInput").ap()
    outp = nc.dram_tensor("outp", [DIM, S], bf16, kind="ExternalOutput").ap()

    r8 = lambda t: t.rearrange("(kk p) n -> p kk n", p=128)

    with tile.TileContext(nc) as tc:
        with tc.tile_pool(name="persist", bufs=1) as pp:
            # Tiles; DMA issue order is arranged inside the body so the
            # first x chunk leads the scalar queue and weights lead SP.
            bq_sb = pp.tile([128, 4], f32)
            ident_sb = pp.tile([128, 128], bf16)
            wqk8_sb = pp.tile([128, 8, 512], f8)
            wqkr8_sb = pp.tile([128, 8, 512], f8)
            wv8_sb = pp.tile([128, 8, 256], f8)
            wvr8_sb = pp.tile([128, 8, 256], f8)
            maskT_sb = pp.tile([128, E_COLS], bf16)
            wo_sb2 = pp.tile([128, 2, DIM], bf16)

            nc.sync.dma_start(out=wqk8_sb[:, 0:4, :], in_=r8(wqk8d)[:, 0:4, :])
            nc.sync.dma_start(out=wqk8_sb[:, 4:8, :], in_=r8(wqk8d)[:, 4:8, :])
            nc.sync.dma_start(out=wqkr8_sb[:, 0:4, :], in_=r8(wqkr8d)[:, 0:4, :])
            nc.sync.dma_start(out=wv8_sb, in_=r8(wv8d))
            nc.sync.dma_start(out=wqkr8_sb[:, 4:8, :], in_=r8(wqkr8d)[:, 4:8, :])
            nc.sync.dma_start(out=wvr8_sb, in_=r8(wvr8d))

            qk_sb = [pp.tile([128, S], bf16, tag=f"qk{m}", name=f"qk{m}")
                     for m in range(4)]
            V65 = pp.tile([128, NT, HPC, 65], bf16, tag="V65", name="V65")
            E_st = pp.tile([128, HPC, E_COLS], bf16, tag="Est", name="Est")
            attn_outT = pp.tile([128, 2, S], bf16, tag="aout", name="aout")
            nc.gpsimd.memset(V65[:, :, :, 64:65], 1.0)

            loop_cm = tc.For_i(0, loop_n, 1) if loop_n > 1 else nullcontext()
            with loop_cm:
                with tc.tile_pool(name="xt_pool", bufs=2) as pax, \
                     tc.tile_pool(name="st_pool", bufs=5) as pst, \
                     tc.tile_pool(name="rec_pool", bufs=3) as prc, \
                     tc.tile_pool(name="at_pool", bufs=4) as pat, \
                     tc.tile_pool(name="psB", bufs=6, space="PSUM") as psb, \
                     tc.tile_pool(name="psPV", bufs=1, space="PSUM") as pspv, \
                     tc.tile_pool(name="psTR", bufs=1, space="PSUM") as pstr:

                    po4_hold = {}
                    tr_hold = {}

                    # warmups during the initial DMA window: ramp the PE
                    # p-state and pull LoadActFuncSet off the hot path
                    dummy_sb = pp.tile([128, 512], bf16, tag="warm",
                                       name="warm")
                    nc.gpsimd.memset(dummy_sb, 0.0)
                    nc.scalar.activation(out=dummy_sb[0:1, 0:1],
                                         in_=dummy_sb[0:1, 1:2], func=AF.Exp)
                    wps = pspv.tile([128, 512], f32, tag="po", name="warmps")
                    for wi in range(7):
                        _lbl("warm")
                        nc.tensor.matmul(wps, dummy_sb[:, 0:128], dummy_sb,
                                         start=(wi == 0), stop=(wi == 6),
                                         skip_group_check=True)

                    def emit_A_dma(n):
                        x8t = pax.tile([128, 8, 512], f8, tag="x8", name=f"x8_{n}")
                        xr8t = pax.tile([128, 8, 512], f8, tag="xr8",
                                        name=f"xr8_{n}")
                        q_eng = nc.scalar if n == 0 else nc.sync
                        if n == 0:
                            # halves, in first-use order, so the term-major
                            # A(0) matmuls chase the DMA stream
                            q_eng.dma_start(out=x8t[:, 0:4, :],
                                            in_=r8(x8d)[:, 0:4, 0:512])
                            q_eng.dma_start(out=xr8t[:, 0:4, :],
                                            in_=r8(xr8d)[:, 0:4, 0:512])
                            q_eng.dma_start(out=x8t[:, 4:8, :],
                                            in_=r8(x8d)[:, 4:8, 0:512])
                            q_eng.dma_start(out=xr8t[:, 4:8, :],
                                            in_=r8(xr8d)[:, 4:8, 0:512])
                        else:
                            q_eng.dma_start(out=x8t,
                                            in_=r8(x8d)[:, :, n * 512:(n + 1) * 512])
                            q_eng.dma_start(out=xr8t,
                                            in_=r8(xr8d)[:, :, n * 512:(n + 1) * 512])
                        if n == 0:
                            nc.scalar.dma_start(
                                out=bq_sb,
                                in_=bqkd.rearrange("(m p) o -> p (m o)", p=128))
                        if n == 1:
                            nc.sync.dma_start(out=maskT_sb, in_=maskTd)
                            nc.sync.dma_start(out=ident_sb, in_=identd)
                        if n == 3:
                            nc.sync.dma_start(
                                out=wo_sb2,
                                in_=wod.rearrange("(p2 p) n -> p p2 n", p=128))
                        return x8t, xr8t

                    xh = {}

                    def emit_A_qk(n, m):
                        x8t, xr8t = xh[n]
                        terms_qk = ((wqk8_sb, x8t), (wqk8_sb, xr8t),
                                    (wqkr8_sb, x8t))
                        ps = psb.tile([128, 512], f32, tag="big",
                                      name=f"psqk{m}_{n}")
                        i = 0
                        for wt, xt in terms_qk:
                            for p in range(4):
                                nc.tensor.matmul(
                                    ps,
                                    wt[:, 2 * p:2 * p + 2, m * 128:(m + 1) * 128],
                                    xt[:, 2 * p:2 * p + 2, :],
                                    start=(i == 0), stop=(i == 11),
                                    perf_mode=DR)
                                i += 1
                        if n == 1:
                            nc.scalar.activation(
                                out=qk_sb[m][:, n * 512:(n + 1) * 512],
                                in_=ps, func=AF.Identity,
                                bias=bq_sb[:, m:m + 1], scale=1.0 / WSCALE)
                        else:
                            nc.vector.tensor_scalar(
                                qk_sb[m][:, n * 512:(n + 1) * 512], ps,
                                1.0 / WSCALE, bq_sb[:, m:m + 1],
                                ALU.mult, ALU.add)

                    def emit_A_v(n, ss):
                        x8t, xr8t = xh[n]
                        terms_v = ((x8t, wv8_sb), (xr8t, wv8_sb), (x8t, wvr8_sb))
                        cc = n * 4 + ss
                        ps = psb.tile([128, 512], f32, tag="big",
                                      name=f"psv{cc}")
                        psv = ps[:, 0:256]
                        i = 0
                        for xt, wt in terms_v:
                            for p in range(4):
                                nc.tensor.matmul(
                                    psv,
                                    xt[:, 2 * p:2 * p + 2, ss * 128:(ss + 1) * 128],
                                    wt[:, 2 * p:2 * p + 2, :],
                                    start=(i == 0), stop=(i == 11),
                                    perf_mode=DR, skip_group_check=True)
                                i += 1
                        if n == 1:
                            nc.vector.tensor_scalar(
                                V65[:, cc, :, 0:64],
                                psv.rearrange("p (h d) -> p h d", h=4),
                                1.0 / WSCALE, None, ALU.mult)
                        else:
                            nc.scalar.activation(
                                out=V65[:, cc, :, 0:64],
                                in_=psv.rearrange("p (h d) -> p h d", h=4),
                                func=AF.Copy, scale=1.0 / WSCALE)

                    def emit_A0_termmajor():
                        x8t, xr8t = xh[0]
                        seq = [(0, 0), (0, 1), (1, 0), (1, 1), (2, 0), (2, 1),
                               (0, 2), (0, 3), (1, 2), (1, 3), (2, 2), (2, 3)]
                        pss_ = [psb.tile([128, 512], f32, tag="big",
                                         name=f"psqk{m}_0") for m in range(4)]
                        terms = ((wqk8_sb, x8t), (wqk8_sb, xr8t),
                                 (wqkr8_sb, x8t))
                        _lbl("Aqk0")
                        for si, (ti, p) in enumerate(seq):
                            wt, xt = terms[ti]
                            for m in range(4):
                                nc.tensor.matmul(
                                    pss_[m],
                                    wt[:, 2 * p:2 * p + 2, m * 128:(m + 1) * 128],
                                    xt[:, 2 * p:2 * p + 2, :],
                                    start=(si == 0), stop=(si == 11),
                                    perf_mode=DR)
                        for m in range(4):
                            nc.vector.tensor_scalar(
                                qk_sb[m][:, 0:512], pss_[m],
                                1.0 / WSCALE, bq_sb[:, m:m + 1],
                                ALU.mult, ALU.add)
                        for ss in range(4):
                            emit_A_v(0, ss)

                    def emit_A(n):
                        if n not in xh:
                            xh[n] = emit_A_dma(n)
                        for m in range(4):
                            emit_A_qk(n, m)
                        for ss in range(4):
                            emit_A_v(n, ss)

                    def emit_Ssc(t):
                        """scores + exp + mask for tile t."""
                        e0 = eoff[t] * 128
                        ncols = nch[t] * 128
                        for h in range(HPC):
                            poff = (h % 2) * 64
                            qT = qk_sb[h // 2]
                            kT = qk_sb[2 + h // 2]
                            ps = psb.tile([128, 512], f32, tag="big",
                                          name=f"sc{h}_{t}")
                            for j in range(nch[t]):
                                nc.tensor.matmul(
                                    ps[:, j * 128:(j + 1) * 128],
                                    kT[poff:poff + 64,
                                       (a[t] + j) * 128:(a[t] + j + 1) * 128],
                                    qT[poff:poff + 64, t * 128:(t + 1) * 128],
                                    start=(j == 0), stop=(j == nch[t] - 1),
                                    skip_group_check=True)
                            nc.scalar.activation(
                                out=E_st[:, h, e0:e0 + ncols],
                                in_=ps[:, 0:ncols], func=AF.Exp)
                            eng = nc.vector if h >= 1 else nc.gpsimd
                            eng.tensor_tensor(
                                E_st[:, h, e0:e0 + ncols],
                                E_st[:, h, e0:e0 + ncols],
                                maskT_sb[:, e0:e0 + ncols], ALU.mult)

                    def emit_PV(t):
                        e0 = eoff[t] * 128
                        po4 = pspv.tile([128, 512], f32, tag="po", name=f"po{t}")
                        po4_hold[t] = po4
                        nmm = HPC * nch[t]
                        i = 0
                        for h in range(HPC):
                            for j in range(nch[t]):
                                nc.tensor.matmul(
                                    po4[:, h * 65:h * 65 + 65],
                                    E_st[:, h, e0 + j * 128:e0 + (j + 1) * 128],
                                    V65[:, a[t] + j, h, :],
                                    start=(i == 0), stop=(i == nmm - 1),
                                    skip_group_check=True)
                                i += 1

                    au_hold = {}

                    def emit_N_pre(t):
                        """normalize (DVE) for tile t."""
                        from concourse.bass import AP

                        po4 = po4_hold.pop(t)
                        den = AP(po4.tensor, po4.offset + 64,
                                 [list(po4.ap[0]), [65, 4]])
                        rec = prc.tile([128, 4], f32, tag="rec", name=f"rec{t}")
                        nc.vector.reciprocal(rec, den)
                        au = pat.tile([128, HPC, 64], bf16, tag="at",
                                      name=f"at{t}")
                        au_hold[t] = au
                        un = AP(po4.tensor, po4.offset,
                                [list(po4.ap[0]), [65, 4], [1, 64]])
                        recb = AP(rec.tensor, rec.offset,
                                  [list(rec.ap[0]), [1, 4], [0, 64]])
                        nc.vector.tensor_tensor(au, un, recb, ALU.mult)

                    def emit_N_post(t):
                        """transpose + copy-out for tile t."""
                        au = au_hold.pop(t)
                        tr = pstr.tile([128, 1024], bf16, tag="tr",
                                       name=f"tr{t}")
                        for i in range(2):
                            _lbl(f"TR{t}")
                            nc.tensor.matmul(
                                tr[:, i * 128:(i + 1) * 128],
                                au[:, 2 * i:2 * i + 2, :], ident_sb,
                                is_transpose=True,
                                start=(i == 0), stop=(i == 1),
                                skip_group_check=True)
                        nc.vector.tensor_copy(
                            attn_outT[:, :, t * 128:(t + 1) * 128],
                            tr[:, 0:256].rearrange("p (i q) -> p i q", i=2))

                    def emit_N(t):
                        emit_N_pre(t)
                        emit_N_post(t)

                    outp3 = outp.rearrange("(mm p) n -> p mm n", p=128)

                    def emit_D_cols_g(nb, ci, g, eng):
                        """4-mm piece over a 256-col half-block (tail)."""
                        c0 = nb * 512 + ci * 256
                        st = pst.tile([128, 4, 256], bf16, tag="st3",
                                      name=f"st3_{nb}_{ci}_{g}")
                        for i in range(4):
                            mm = g * 4 + i
                            _lbl(f"D{nb}.{mm}c{ci}")
                            ps = psb.tile([128, 512], f32, tag="big",
                                          name=f"psd{mm}_{nb}c{ci}")
                            psx = ps[:, 0:256]
                            for p2 in range(2):
                                nc.tensor.matmul(
                                    psx,
                                    wo_sb2[:, p2, mm * 128:(mm + 1) * 128],
                                    attn_outT[:, p2, c0:c0 + 256],
                                    start=(p2 == 0), stop=(p2 == 1),
                                    skip_group_check=True)
                            eng(st[:, i, :], psx)
                        q_eng = nc.scalar if (g + ci) % 2 else nc.sync
                        q_eng.dma_start(
                            out=outp3[:, g * 4:(g + 1) * 4, c0:c0 + 256],
                            in_=st)

                    def emit_D_cols2(nb, ci, g, sub):
                        """2-mm tail piece: parallel ACT/DVE copies + store."""
                        c0 = nb * 512 + ci * 256
                        m0 = g * 4 + sub * 2
                        st = pst.tile([128, 2, 256], bf16, tag="st4",
                                      name=f"st4_{nb}_{ci}_{g}_{sub}")
                        for idx in range(2):
                            mm = m0 + idx
                            _lbl(f"D{nb}.{mm}c{ci}")
                            ps = psb.tile([128, 512], f32, tag="big",
                                          name=f"psd{mm}_{nb}c{ci}")
                            psx = ps[:, 0:256]
                            for p2 in range(2):
                                nc.tensor.matmul(
                                    psx,
                                    wo_sb2[:, p2, mm * 128:(mm + 1) * 128],
                                    attn_outT[:, p2, c0:c0 + 256],
                                    start=(p2 == 0), stop=(p2 == 1),
                                    skip_group_check=True)
                            if idx == 0:
                                nc.scalar.copy(st[:, idx, :], psx)
                            else:
                                nc.vector.tensor_copy(st[:, idx, :], psx)
                        q_eng = nc.scalar if sub % 2 else nc.sync
                        q_eng.dma_start(
                            out=outp3[:, m0:m0 + 2, c0:c0 + 256], in_=st)

                    def emit_D_half(nb, half):
                        if nb == 3:
                            emit_D_pair(nb, half, 0)
                            emit_D_pair(nb, half, 1)
                            return
                        st = pst.tile([128, 4, 512], bf16, tag="st",
                                      name=f"st{half}_{nb}")
                        for i in range(4):
                            mm = half * 4 + i
                            _lbl(f"D{nb}.{mm}")
                            ps = psb.tile([128, 512], f32, tag="big",
                                          name=f"psd{mm}_{nb}")
                            for p2 in range(2):
                                nc.tensor.matmul(
                                    ps, wo_sb2[:, p2, mm * 128:(mm + 1) * 128],
                                    attn_outT[:, p2, nb * 512:(nb + 1) * 512],
                                    start=(p2 == 0), stop=(p2 == 1))
                            if (mm + nb) % 2 == 0:
                                nc.scalar.copy(st[:, i, :], ps)
                            else:
                                nc.vector.tensor_copy(st[:, i, :], ps)
                        nc.sync.dma_start(
                            out=outp3[:, half * 4:(half + 1) * 4,
                                      nb * 512:(nb + 1) * 512],
                            in_=st)

                    # software-pipelined emission: PV lags scores by one
                    # step, N by two; A/D matmul pieces fill PE while ACT
                    # chews the exps.
                    xh[0] = emit_A_dma(0)
                    emit_A0_termmajor()
                    emit_A(1)
                    xh[2] = emit_A_dma(2)
                    emit_Ssc(0)
                    emit_Ssc(1); emit_PV(0); emit_A_qk(2, 0)
                    emit_Ssc(2); emit_PV(1); emit_N(0); emit_A_qk(2, 1)
                    emit_N_pre(1); emit_Ssc(3); emit_PV(2); emit_N_post(1); emit_A_qk(2, 2)
                    emit_N_pre(2); emit_Ssc(4); emit_PV(3); emit_N_post(2); emit_A_qk(2, 3)
                    xh[3] = emit_A_dma(3)
                    emit_N_pre(3); emit_Ssc(5); emit_PV(4); emit_N_post(3); emit_A_v(2, 0); emit_A_v(2, 1)
                    emit_N_pre(4); emit_Ssc(6); emit_PV(5); emit_N_post(4); emit_A_v(2, 2); emit_A_v(2, 3)
                    emit_N_pre(5); emit_Ssc(7); emit_PV(6); emit_N_post(5); emit_A_qk(3, 0); emit_A_qk(3, 1)
                    emit_N_pre(6); emit_Ssc(8); emit_PV(7); emit_N_post(6); emit_A_qk(3, 2); emit_A_qk(3, 3)
                    emit_N_pre(7); emit_Ssc(9); emit_PV(8); emit_N_post(7); emit_A_v(3, 0); emit_A_v(3, 1)
                    emit_N_pre(8); emit_Ssc(10); emit_PV(9); emit_N_post(8); emit_A_v(3, 2); emit_A_v(3, 3)
                    emit_N_pre(9); emit_Ssc(11); emit_PV(10); emit_N_post(9); emit_D_half(0, 0)
                    emit_N_pre(10); emit_Ssc(12); emit_PV(11); emit_N_post(10); emit_D_half(0, 1)
                    emit_N_pre(11); emit_Ssc(13); emit_PV(12); emit_N_post(11); emit_D_half(1, 0)
                    emit_N_pre(12); emit_Ssc(14); emit_PV(13); emit_N_post(12); emit_D_half(1, 1)
                    emit_N_pre(13); emit_Ssc(15); emit_PV(14); emit_N_post(13); emit_D_half(2, 0)
                    emit_PV(15); emit_D_half(2, 1)
                    emit_N(14)
                    emit_D_cols_g(3, 0, 0, lambda o, i: nc.scalar.copy(o, i))
                    emit_N(15)
                    emit_D_cols_g(3, 0, 1, lambda o, i: nc.vector.tensor_copy(o, i))
                    emit_D_cols_g(3, 1, 0, lambda o, i: nc.scalar.copy(o, i))
                    emit_D_cols2(3, 1, 1, 0)
                    emit_D_cols2(3, 1, 1, 1)

    nc.compile()
    return nc


def _get_module(a, nch, eoff):
    key = (tuple(int(v) for v in a), tuple(int(v) for v in nch))
    if key not in _CACHE:
        _CACHE[key] = _build_module(a, nch, eoff)
    return _CACHE[key]


def _split8(arr):
    """x -> (x8, xr8) e4m3 pair with x ~= x8 + xr8 (residual hits denormals)."""
    import ml_dtypes

    f8 = ml_dtypes.float8_e4m3
    a = np.asarray(arr, np.float32)
    hi = a.astype(f8)
    lo = (a - hi.astype(np.float32)).astype(f8)
    return hi, lo


def _prepare_in_maps(x, routes, qkv_w, qkv_b, out_w, out_b):
    import ml_dtypes

    bf = ml_dtypes.bfloat16
    x = np.ascontiguousarray(np.asarray(x, np.float32))
    qkv_w = np.asarray(qkv_w, np.float32)
    qkv_b = np.asarray(qkv_b, np.float32)
    out_w = np.asarray(out_w, np.float32)

    pi, rank, kr, a, nch, eoff = _geometry(np.asarray(routes))
    E_CH = int(eoff[-1] + nch[-1])
    E_COLS = E_CH * 128
    SCALE = 1.0 / float(np.sqrt(DH))

    # maskT [128, E_COLS]: maskT[p, (eoff[t]+j)*128 + q] = 1 iff key
    # (a[t]+j)*128+p is routed for query t*128+q
    maskT_np = np.zeros((128, E_COLS), np.float32)
    for t in range(NT):
        krt = kr[t * QT:(t + 1) * QT] - a[t] * 128          # [128, K]
        qi = np.repeat(np.arange(QT), K_NEI)
        kk = krt.ravel()
        j, p = kk // 128, kk % 128
        maskT_np[p, (eoff[t] + j) * 128 + qi] = 1.0
    maskT_np = maskT_np.astype(bf)

    ident_np = np.eye(128, dtype=np.float32).astype(bf)

    x8_b, xr8_b = [], []
    for b in range(B):
        x8, xr8 = _split8(x[b][pi].T)
        x8_b.append(np.ascontiguousarray(x8))
        xr8_b.append(np.ascontiguousarray(xr8))

    in_maps = []
    for c in range(N_CORES):
        b = c // (N_CORES // B)
        hb = c % (N_CORES // B)
        heads = range(hb * HPC, (hb + 1) * HPC)
        w_cols, b_rows = [], []
        for sect, scale in ((0, SCALE), (1, 1.0)):
            for h in heads:
                r0 = sect * DIM + h * DH
                w_cols.append(qkv_w[r0:r0 + DH] * scale)
                b_rows.append(qkv_b[r0:r0 + DH] * scale)
        wqk = np.concatenate(w_cols, 0).T * WSCALE           # [1024, 512]
        wqk8, wqkr8 = _split8(wqk)
        bqk_c = np.concatenate(b_rows, 0).reshape(-1, 1).astype(np.float32)
        vr0 = 2 * DIM + hb * HPC * DH
        wv = qkv_w[vr0:vr0 + 256].T * WSCALE                 # [1024, 256]
        wv8, wvr8 = _split8(wv)
        wo_c = np.ascontiguousarray(
            out_w[:, hb * HPC * DH:(hb + 1) * HPC * DH].T).astype(bf)
        in_maps.append({
            "x8": x8_b[b], "xr8": xr8_b[b],
            "wqk8": np.ascontiguousarray(wqk8),
            "wqkr8": np.ascontiguousarray(wqkr8),
            "wv8": np.ascontiguousarray(wv8),
            "wvr8": np.ascontiguousarray(wvr8),
            "bqk": bqk_c,
            "wo": wo_c,
            "maskT": maskT_np,
            "ident": ident_np,
        })
    return in_maps, pi, (a, nch, eoff)


def kernel(x, routes, qkv_w, qkv_b, out_w, out_b):
    from concourse.bass_utils import run_bass_kernel_spmd

    out_b = np.asarray(out_b, np.float32)
    qkv_b = np.asarray(qkv_b, np.float32)
    out_w = np.asarray(out_w, np.float32)
    in_maps, pi, geom = _prepare_in_maps(x, routes, qkv_w, qkv_b, out_w, out_b)

    nc = _get_module(*geom)
    res = run_bass_kernel_spmd(nc, in_maps, core_ids=list(range(N_CORES)))

    # v-bias: probs sum to 1 -> attn@(v+bv) = attn@v + bv; outp picks up a
    # constant wo.T @ bv per core -- add on host (in the kernel's bf16 wo).
    bv_all = qkv_b[2 * DIM:3 * DIM]
    adj = np.zeros(DIM, np.float64)
    for hb in range(N_CORES // B):
        sl = slice(hb * HPC * DH, (hb + 1) * HPC * DH)
        wo_c = np.asarray(in_maps[hb]["wo"], np.float64)     # [256, DIM]
        adj += bv_all[sl].astype(np.float64) @ wo_c

    out = np.empty((B, S, DIM), np.float32)
    for b in range(B):
        cores = [c for c in range(N_CORES) if c // (N_CORES // B) == b]
        outT = res.results[cores[0]]["outp"].astype(np.float32)
        for c in cores[1:]:
            outT = outT + res.results[c]["outp"].astype(np.float32)
        rows_sorted = outT.T                      # [S, DIM] in rank order
        tmp = np.empty_like(rows_sorted)
        tmp[pi] = rows_sorted
        out[b] = tmp + (out_b.astype(np.float64) + adj)[None, :].astype(np.float32)
    return out
